# revision 48
# baseline (speedup 1.0000x reference)
"""Trainium2 Bass kernel for nn_Network_33320356282492 (dense_cnn).

Data-parallel over 8 NeuronCores: 2 samples/core. Per core the 6
(sample, channel) planes are split into 16 row-groups giving a
[96, 16384] on-chip layout (partition=(b,c,g), free=32 rows x 512 cols),
processed in 16 column-chunks of 1024.

All matmuls run in float32r (single-pass reduced-precision fp32 on the
PE: 4x the fp32 rate, ~1.4e-4 rel err; storage bits are plain f32).

Conv layer = one PE matmul (block-diagonal 3x3 channel mix with the
residual identity folded in) + one ScalarE Prelu whose per-partition
bias/alpha APs carry the conv bias and leaky slope. Conv is emitted
LAYER-MAJOR over super-blocks of 2 chunks so the in-order PE stream
never parks a ready matmul behind one still waiting on a prelu; ScalarE
then runs prelus back-to-back (it is the busiest engine).

Polynomial attention: out = (P' + x*(Q + x*(R + c12*x))) * x, where
P/Q/R are per-pixel polynomial planes computed BY THE PE from 10
monomial basis planes in (hf, w); the per-rowgroup h offset is folded
into per-partition coefficients on the host. The cubic term rides the
R matmul as a diagonal block over x (the layer-3 prelu writes x4 back
into the input tile so x4 and the basis live in one K=107 AP), and the
two Horner additions are PE identity-matmul accumulations into PSUM —
only 3 elementwise DVE multiplies per 512-lane remain.
"""
import numpy as np
import concourse.bass as bass
import concourse.bacc as bacc
import concourse.mybir as mybir
import concourse.tile as tile
from concourse.bass_utils import run_bass_kernel_spmd

F32 = mybir.dt.float32
F32R = mybir.dt.float32r
AF = mybir.ActivationFunctionType
ALU = mybir.AluOpType


def _f(ap):
    """View a float32r AP as plain f32 for non-matmul consumers (same bits)."""
    return ap.bitcast(F32)

NCORES = 8
BPC = 2                    # samples per core
G = 16                     # row-groups per plane
P = 96                     # active partitions = BPC*3*G
F = 16384                  # free elements per partition (32 rows x 512 cols)
WOFF = F                   # weights start column in the blob
# weight-area columns (offsets relative to WOFF)
WCONV = 0                  # 4 x [96, 96]
WRX = 384                  # [96, 96] diag c12
WP = 480                   # [10, 96]
WQ = 576                   # [6, 96]
WR = 672                   # [3, 96]
WSLOPE = 768               # [96, 4] leaky slopes (Prelu alpha)
WBIAS = 772                # [96, 4] conv biases (Prelu bias)
WID = 776                  # [96, 96] identity (PE-side Horner adds)
EXTRA = 896
FT = F + EXTRA

CHUNK = 1024
NCH = F // CHUNK
SB = 2                     # chunks per super-block (layer-major conv)
A = 2.0 / 511.0            # coordinate scale: h/w = A*idx - 1


def _build_blob(xs, feat):
    """xs: [2,3,512,512] f32, feat: [2,120] f32 -> blob [128, FT] f32."""
    blob = np.zeros((128, FT), dtype=np.float32)
    blob[0:P, 0:F] = xs.reshape(6, G, F).reshape(P, F)

    # basis planes (partitions 97..106): monomials in hf (row-in-group) and w
    rl = (A * np.arange(32, dtype=np.float64))[:, None] * np.ones((1, 512))
    wv = (A * np.arange(512, dtype=np.float64) - 1.0)[None, :] * np.ones((32, 1))
    rows = [np.ones_like(rl), rl, wv, rl * rl, wv * wv, rl * wv,
            rl ** 3, wv ** 3, rl * rl * wv, rl * wv * wv]
    blob[P + 1:P + 11, 0:F] = np.stack([r.reshape(F) for r in rows]).astype(np.float32)

    params = feat[:, :60].reshape(BPC, 4, 15).astype(np.float64)
    coef = feat[:, 60:].reshape(BPC, 20, 3).astype(np.float64)

    # conv layers: lhsT [96, 96] with residual identity folded in; bias and
    # slope ride the Prelu activation as per-partition APs
    for layer in range(4):
        pl = params[:, layer]                       # [2, 15]
        k = pl[:, 0:9].reshape(BPC, 3, 3)           # [b, out, in]
        kp = k + np.eye(3)[None]
        bias = pl[:, 9:12]
        slope = pl[:, 12:15]
        Wm = np.zeros((96, 96))
        for b in range(BPC):
            for co in range(3):
                cols = (b * 3 + co) * G + np.arange(G)
                for ci in range(3):
                    Wm[(b * 3 + ci) * G + np.arange(G), cols] = kp[b, co, ci]
        blob[0:96, WOFF + WCONV + layer * 96: WOFF + WCONV + (layer + 1) * 96] = Wm
        for b in range(BPC):
            for c in range(3):
                rows_p = (b * 3 + c) * G + np.arange(G)
                blob[rows_p, WOFF + WSLOPE + layer] = slope[b, c]
                blob[rows_p, WOFF + WBIAS + layer] = bias[b, c]

    # per-partition h offset g0 = A*(g*32) - 1
    g0 = np.zeros(96)
    c12 = np.zeros(96)
    cc = {}
    for b in range(BPC):
        for c in range(3):
            for g in range(G):
                p = (b * 3 + c) * G + g
                g0[p] = A * (g * 32) - 1.0
                cc[p] = coef[b, :, c]
                c12[p] = coef[b, 12, c]

    wp = np.zeros((10, 96))
    wq = np.zeros((6, 96))
    wr = np.zeros((3, 96))
    for p in range(96):
        c = cc[p]
        t = g0[p]
        # P' (x^0 of att+1): h^a w^b coefficients
        p00, p10, p01 = c[0] + 1.0, c[1], c[2]
        p20, p02, p11 = c[4], c[5], c[7]
        p30, p03, p21, p12 = c[10], c[11], c[13], c[16]
        wp[:, p] = [
            p00 + p10 * t + p20 * t * t + p30 * t ** 3,   # 1
            p10 + 2 * p20 * t + 3 * p30 * t * t,          # hf
            p01 + p11 * t + p21 * t * t,                  # w
            p20 + 3 * p30 * t,                            # hf^2
            p02 + p12 * t,                                # w^2
            p11 + 2 * p21 * t,                            # hf*w
            p30, p03, p21, p12,                           # hf^3, w^3, hf^2 w, hf w^2
        ]
        # Q (x^1)
        q00, q10, q01, q20, q02, q11 = c[3], c[8], c[9], c[14], c[15], c[19]
        wq[:, p] = [
            q00 + q10 * t + q20 * t * t,
            q10 + 2 * q20 * t,
            q01 + q11 * t,
            q20, q02, q11,
        ]
        # R (x^2)
        r00, r10, r01 = c[6], c[17], c[18]
        wr[:, p] = [r00 + r10 * t, r10, r01]

    # pqr coefficients live on the BASIS partitions (97..106) so the matmuls
    # can use base-64 APs (rows 64..96 of the lhsT are zero => x rows and the
    # unused ones-row contribute nothing)
    blob[0:96, WOFF + WRX: WOFF + WRX + 96] = np.diag(c12)
    blob[97:100, WOFF + WRX: WOFF + WRX + 96] = wr
    blob[0:96, WOFF + WID: WOFF + WID + 96] = np.eye(96)
    blob[97:107, WOFF + WP: WOFF + WP + 96] = wp
    blob[97:103, WOFF + WQ: WOFF + WQ + 96] = wq
    return blob


def _build_bass():
    # Bacc (not raw Bass): its compile() pass splits multi-semaphore waits
    # into event-semaphore chains — the HW allows only 1 sync wait per
    # instruction on most engines
    nc = bacc.Bacc()
    # the blob is declared float32r: PE matmuls then run single-pass (4x
    # faster than true fp32, ~1.4e-4 rel err); bits are plain f32
    blob = nc.declare_dram_parameter("blob", [128, FT], F32R, isOutput=False)
    y_ext = nc.declare_dram_parameter("y", [P, F], F32, isOutput=True)

    with tile.TileContext(nc) as tc:
        with (
            tc.tile_pool(name="cwp", bufs=1) as cwp,
            tc.tile_pool(name="xin", bufs=8) as xin,
            tc.tile_pool(name="ping", bufs=8) as ping,
            tc.tile_pool(name="scr", bufs=6) as scr,
            tc.tile_pool(name="outp", bufs=6) as outp,
            tc.tile_pool(name="psy", bufs=2, space="PSUM") as psy,
            tc.tile_pool(name="psq", bufs=4, space="PSUM") as psq,
        ):
            cw = cwp.tile([128, EXTRA], F32R)
            nc.sync.dma_start(out=cw[:], in_=blob[0:128, WOFF:FT])

            sb_state = {}

            def emit_conv_sb(sb):
                """Layer-major conv over a super-block of SB chunks: the PE
                never queues a matmul behind one that waits on a prelu of the
                same chunk, so ScalarE runs prelus back-to-back."""
                js = list(range(sb * SB, (sb + 1) * SB))
                xts, cur = {}, {}
                for j in js:
                    cs = slice(j * CHUNK, (j + 1) * CHUNK)
                    xt = xin.tile([107, CHUNK], F32R, tag="xt")
                    nc.sync.dma_start(out=xt[:], in_=blob[0:107, cs])
                    xts[j] = xt
                    cur[j] = xt
                for layer in range(4):
                    c0 = WCONV + layer * 96
                    alpha = _f(cw[0:96, WSLOPE + layer: WSLOPE + layer + 1])
                    bias = _f(cw[0:96, WBIAS + layer: WBIAS + layer + 1])
                    for j in js:
                        yps = psy.tile([96, CHUNK], F32, tag="y")
                        for s in range(CHUNK // 512):
                            ss = slice(s * 512, (s + 1) * 512)
                            nc.tensor.matmul(yps[:, ss], cw[0:96, c0:c0 + 96],
                                             cur[j][0:96, ss],
                                             start=True, stop=True)
                        if layer == 3:
                            # x4 lands in the xt tile's x rows (dead since the
                            # layer-0 matmul): R' then needs ONE K=107 matmul
                            nxt = xts[j][0:96, 0:CHUNK]
                        else:
                            nxt = ping.tile([96, CHUNK], F32R, tag="pp")
                        nc.scalar.activation(nxt[:], yps[:], AF.Prelu,
                                             bias=bias, scale=1.0, alpha=alpha)
                        cur[j] = nxt
                sb_state[sb] = xts

            def emit_poly_sb(sb):
                """Step-major polynomial over the super-block: each PE step
                consumes DVE results produced a full step earlier."""
                js = list(range(sb * SB, (sb + 1) * SB))
                xts = sb_state.pop(sb)
                lanes = [(j, s) for j in js for s in range(CHUNK // 512)]
                rps, t1s, qps, t3s, pps, ots = {}, {}, {}, {}, {}, {}
                for j in js:
                    ots[j] = outp.tile([96, CHUNK], F32, tag="out", name="ot")
                for j, s in lanes:
                    ss = slice(s * 512, (s + 1) * 512)
                    ps = psq.tile([96, 512], F32, tag="pqr")
                    nc.tensor.matmul(ps[:], cw[0:107, WRX:WRX + 96],
                                     xts[j][0:107, ss], start=True, stop=True)
                    rps[(j, s)] = ps
                for j, s in lanes:
                    ss = slice(s * 512, (s + 1) * 512)
                    t1 = scr.tile([96, 512], F32R, tag="t1")
                    nc.vector.tensor_tensor(out=t1[:], in0=_f(xts[j][0:96, ss]),
                                            in1=rps[(j, s)][:], op=ALU.mult)
                    t1s[(j, s)] = t1
                for j, s in lanes:
                    ss = slice(s * 512, (s + 1) * 512)
                    ps = psq.tile([96, 512], F32, tag="pqr")
                    nc.tensor.matmul(ps[:], cw[64:107, WQ:WQ + 96],
                                     xts[j][64:107, ss], start=True, stop=False)
                    nc.tensor.matmul(ps[:], cw[0:96, WID:WID + 96],
                                     t1s[(j, s)][0:96, :], start=False, stop=True)
                    qps[(j, s)] = ps
                for j, s in lanes:
                    ss = slice(s * 512, (s + 1) * 512)
                    t3 = scr.tile([96, 512], F32R, tag="t3")
                    nc.vector.tensor_tensor(out=t3[:], in0=_f(xts[j][0:96, ss]),
                                            in1=qps[(j, s)][:], op=ALU.mult)
                    t3s[(j, s)] = t3
                for j, s in lanes:
                    ss = slice(s * 512, (s + 1) * 512)
                    ps = psq.tile([96, 512], F32, tag="pqr")
                    nc.tensor.matmul(ps[:], cw[64:107, WP:WP + 96],
                                     xts[j][64:107, ss], start=True, stop=False)
                    nc.tensor.matmul(ps[:], cw[0:96, WID:WID + 96],
                                     t3s[(j, s)][0:96, :], start=False, stop=True)
                    pps[(j, s)] = ps
                for j, s in lanes:
                    ss = slice(s * 512, (s + 1) * 512)
                    nc.vector.tensor_tensor(out=ots[j][:, ss],
                                            in0=_f(xts[j][0:96, ss]),
                                            in1=pps[(j, s)][:], op=ALU.mult)
                for j in js:
                    cs = slice(j * CHUNK, (j + 1) * CHUNK)
                    nc.sync.dma_start(out=y_ext[:, cs], in_=ots[j][:])

            NSB = NCH // SB
            for sb in range(NSB + 1):
                if sb < NSB:
                    emit_conv_sb(sb)
                if sb >= 1:
                    emit_poly_sb(sb - 1)
    nc.finalize()
    return nc


def kernel(x, feature):
    x = np.ascontiguousarray(x, dtype=np.float32)
    feature = np.ascontiguousarray(feature, dtype=np.float32)
    nc = _build_bass()
    in_maps = [
        {"blob": _build_blob(x[2 * cb:2 * cb + 2], feature[2 * cb:2 * cb + 2])}
        for cb in range(NCORES)
    ]
    res = run_bass_kernel_spmd(nc, in_maps, list(range(NCORES)))
    out = np.empty((16, 3, 512, 512), dtype=np.float32)
    for cb in range(NCORES):
        y = res.results[cb]["y"]
        out[2 * cb:2 * cb + 2] = y.reshape(BPC, 3, G, 32, 512).reshape(BPC, 3, 512, 512)
    return out
